# revision 5
# baseline (speedup 1.0000x reference)
"""Trainium2 Bass kernel for nn_LocalResiduals (locally-connected 3x3 stencil + MLP).

Sharding: 8 cores x 2048 pixels (npix-parallel, per sharding hint).
Per-core device kernel:
  part1: per-pixel matmul pairs on TensorE:
     out_p(16m,16b) = W_main_p(128kn,16m)^T @ X_main_p(128kn,16b)   [k=0..7]
                    + W_cent_p(16n,16m)^T  @ X_cent_p(16n,16b)      [k=8]
  part2: shared MLP  h=relu(W1@[inter;noise2]+b1); out=W2@h+b2  (fp32)
Host does gather/layout prep (bf16 cast for part1 operands).
"""
import sys
import os

sys.path.insert(0, "/opt/trn_rl_repo")

import numpy as np
import ml_dtypes

H, W, NF, K, MD, ND, NDM, MLP_H = 128, 128, 8, 9, 16, 8, 8, 64
NPIX = H * W
B = 16
NIN = NF + ND  # 16
NCORES = 8
PPC = NPIX // NCORES  # 2048 pixels per core
CHUNK = 256           # pixels per on-device chunk
NCHUNK = PPC // CHUNK
TOK = CHUNK * B       # 4096 tokens per chunk
D0 = MD + NDM         # 24

_BF16 = ml_dtypes.bfloat16


def _patch_tile_drain():
    """walrus CoreV3 rejects >2 sync-waits on a CTRL (Drain) instruction.
    Tile's tail drain carries one wait per outstanding proc sem; split the
    excess onto extra drain instructions."""
    import concourse.tile as tile
    from concourse.tile import ScopedClock

    if getattr(tile.TileContext, "_drain_patched", False):
        return

    def _drain_and_barrier(self, tick_clock, wait_clock):
        nc = self.nc
        drain_inst = nc.sync.drain()
        wait_clock.add_sem_waits(
            drain_inst.ins, ScopedClock({None: tick_clock.global_clock})
        )
        si = drain_inst.ins.sync_info
        if si is not None and si.on_wait and len(si.on_wait) > 2:
            waits = list(si.on_wait)
            si.on_wait = waits[:2]
            rest = waits[2:]
            while rest:
                extra = nc.sync.drain()
                esi = extra.ins.sync_info
                if esi is None:
                    import concourse.mybir as mybir

                    extra.ins.sync_info = mybir.SyncInfo(
                        on_wait=rest[:2], on_update=[]
                    )
                else:
                    esi.on_wait = rest[:2]
                rest = rest[2:]

        nc.all_engine_barrier()
        assert self.sems is not None
        popped = nc._tile_sem_poison_stack.pop()
        assert popped is self._sem_poison
        nc.clear_and_free_semaphores(list(self.sems.allocated().values()))
        nc.all_engine_barrier()

    tile.TileContext._drain_and_barrier = _drain_and_barrier
    tile.TileContext._drain_patched = True


def _split_sync_waits(nc, mybir, limit=1):
    """walrus CoreV3 accepts at most `limit` sync waits per instruction.
    Hoist excess waits onto same-engine nops inserted just before."""

    def _find_and_remove(inst):
        for f in nc.m.functions:
            for bb in f.blocks:
                il = bb.instructions
                for i, x in enumerate(il):
                    if x.name == inst.name:
                        del il[i]
                        bb.instructions = il
                        return

    for f in nc.m.functions:
        for bb in f.blocks:
            il = bb.instructions
            out = []
            changed = False
            for inst in il:
                si = inst.sync_info
                if si is not None and si.on_wait and len(si.on_wait) > limit:
                    waits = list(si.on_wait)
                    head, tail = waits[:-limit], waits[-limit:]
                    for j in range(0, len(head), limit):
                        nop = nc.engines[inst.engine].nop(nofuse=True)
                        _find_and_remove(nop.ins)
                        nop.ins.sync_info = mybir.SyncInfo(
                            on_wait=head[j : j + limit], on_update=[]
                        )
                        out.append(nop.ins)
                    si.on_wait = tail
                    changed = True
                out.append(inst)
            if changed:
                bb.instructions = out


def _build_program():
    import concourse.bass as bass
    import concourse.tile as tile
    from concourse import mybir

    _patch_tile_drain()

    nc = bass.Bass()
    dt = mybir.dt
    PXF = PPC * MD  # 32768 = pixel-major free size (16 cols per px)

    wm = nc.declare_dram_parameter("wm", [128, PXF], dt.bfloat16, isOutput=False)
    xm = nc.declare_dram_parameter("xm", [128, PXF], dt.bfloat16, isOutput=False)
    wc = nc.declare_dram_parameter("wc", [16, PXF], dt.bfloat16, isOutput=False)
    xc = nc.declare_dram_parameter("xc", [16, PXF], dt.bfloat16, isOutput=False)
    nz = nc.declare_dram_parameter("nz", [8, PPC * B], dt.float32, isOutput=False)
    w1t = nc.declare_dram_parameter("w1t", [D0, MLP_H], dt.float32, isOutput=False)
    b1 = nc.declare_dram_parameter("b1", [MLP_H, 1], dt.float32, isOutput=False)
    w2t = nc.declare_dram_parameter("w2t", [MLP_H, NF], dt.float32, isOutput=False)
    b2 = nc.declare_dram_parameter("b2", [NF, 1], dt.float32, isOutput=False)
    yout = nc.declare_dram_parameter("yout", [NF, PPC * B], dt.float32, isOutput=True)

    CF = CHUNK * MD  # free cols per chunk in wm/xm (4096)

    with tile.TileContext(nc) as tc:
        with (
            tc.tile_pool(name="consts", bufs=1) as cpool,
            tc.tile_pool(name="wx", bufs=3) as wxpool,
            tc.tile_pool(name="mlp", bufs=2) as mlppool,
            tc.tile_pool(name="outp", bufs=2) as outpool,
            tc.tile_pool(name="ps1", bufs=4, space="PSUM") as ps1pool,
            tc.tile_pool(name="ps2", bufs=2, space="PSUM") as ps2pool,
            tc.tile_pool(name="ps3", bufs=2, space="PSUM") as ps3pool,
        ):
            w1_t = cpool.tile([D0, MLP_H], dt.float32, tag="w1")
            nc.sync.dma_start(w1_t[:], w1t[:])
            b1_t = cpool.tile([MLP_H, 1], dt.float32, tag="b1")
            nc.sync.dma_start(b1_t[:], b1[:])
            w2_t = cpool.tile([MLP_H, NF], dt.float32, tag="w2")
            nc.sync.dma_start(w2_t[:], w2t[:])
            b2_t = cpool.tile([NF, 1], dt.float32, tag="b2")
            nc.sync.dma_start(b2_t[:], b2[:])

            for ch in range(NCHUNK):
                cs = slice(ch * CF, (ch + 1) * CF)
                wm_t = wxpool.tile([128, CF], dt.bfloat16, tag="wm")
                nc.sync.dma_start(wm_t[:], wm[:, cs])
                xm_t = wxpool.tile([128, CF], dt.bfloat16, tag="xm")
                nc.sync.dma_start(xm_t[:], xm[:, cs])
                wc_t = wxpool.tile([16, CF], dt.bfloat16, tag="wc")
                nc.sync.dma_start(wc_t[:], wc[:, cs])
                xc_t = wxpool.tile([16, CF], dt.bfloat16, tag="xc")
                nc.sync.dma_start(xc_t[:], xc[:, cs])

                mlp_in = mlppool.tile([D0, TOK], dt.float32, tag="mlpin")
                nc.sync.dma_start(
                    mlp_in[MD:D0, :], nz[:, ch * TOK : (ch + 1) * TOK]
                )

                # part 1: per-pixel contraction, 32 px per PSUM bank
                for g in range(CHUNK // 32):
                    ps = ps1pool.tile([16, 512], dt.float32, tag="p1")
                    for s in range(32):
                        px = g * 32 + s
                        c16 = slice(px * 16, (px + 1) * 16)
                        o16 = slice(s * 16, (s + 1) * 16)
                        nc.tensor.matmul(
                            out=ps[:, o16],
                            lhsT=wm_t[:, c16],
                            rhs=xm_t[:, c16],
                            start=True,
                            stop=False,
                        )
                        nc.tensor.matmul(
                            out=ps[:, o16],
                            lhsT=wc_t[:, c16],
                            rhs=xc_t[:, c16],
                            start=False,
                            stop=True,
                        )
                    if g % 2 == 0:
                        nc.vector.tensor_copy(
                            mlp_in[0:MD, g * 512 : (g + 1) * 512], ps[:]
                        )
                    else:
                        nc.scalar.activation(
                            mlp_in[0:MD, g * 512 : (g + 1) * 512], ps[:],
                            mybir.ActivationFunctionType.Copy,
                        )

                # part 2: MLP over 4096 tokens
                h_sb = mlppool.tile([MLP_H, TOK], dt.float32, tag="h")
                for t in range(TOK // 512):
                    t512 = slice(t * 512, (t + 1) * 512)
                    hps = ps2pool.tile([MLP_H, 512], dt.float32, tag="hps")
                    nc.tensor.matmul(
                        out=hps[:], lhsT=w1_t[:], rhs=mlp_in[:, t512],
                        start=True, stop=True,
                    )
                    nc.scalar.activation(
                        h_sb[:, t512], hps[:],
                        mybir.ActivationFunctionType.Relu,
                        bias=b1_t[:, 0:1],
                    )
                o_sb = outpool.tile([NF, TOK], dt.float32, tag="osb")
                for t in range(TOK // 512):
                    t512 = slice(t * 512, (t + 1) * 512)
                    ops = ps3pool.tile([NF, 512], dt.float32, tag="ops")
                    nc.tensor.matmul(
                        out=ops[:], lhsT=w2_t[:], rhs=h_sb[:, t512],
                        start=True, stop=True,
                    )
                    nc.vector.tensor_tensor(
                        out=o_sb[:, t512],
                        in0=ops[:],
                        in1=b2_t[:, 0:1].to_broadcast([NF, 512]),
                        op=mybir.AluOpType.add,
                    )
                nc.sync.dma_start(yout[:, ch * TOK : (ch + 1) * TOK], o_sb[:])

    _split_sync_waits(nc, mybir)
    return nc


_NC_CACHE = None


def _get_nc():
    global _NC_CACHE
    if _NC_CACHE is None:
        _NC_CACHE = _build_program()
    return _NC_CACHE


# test.py can set this to capture profile info
LAST_RESULTS = None
TRACE = bool(os.environ.get("BASS_KERNEL_TRACE"))


def kernel(y_in, noise, noise2, weight_map, w1, b1, w2, b2, neighbor_idx):
    from concourse.bass_utils import run_bass_kernel_spmd

    y_in = np.asarray(y_in, np.float32)
    noise = np.asarray(noise, np.float32)
    noise2 = np.asarray(noise2, np.float32)
    weight_map = np.asarray(weight_map, np.float32)
    w1 = np.asarray(w1, np.float32)
    b1v = np.asarray(b1, np.float32)
    w2 = np.asarray(w2, np.float32)
    b2v = np.asarray(b2, np.float32)
    nbr = np.asarray(neighbor_idx)

    feats = np.concatenate([y_in.reshape(B, NF, NPIX), noise], axis=1)  # (B,16,NPIX)
    G = np.ascontiguousarray(feats.transpose(2, 1, 0))  # (NPIX, 16n, 16b)

    w1t_np = np.ascontiguousarray(w1.T)          # (24, 64)
    b1_np = b1v.reshape(MLP_H, 1)
    w2t_np = np.ascontiguousarray(w2.T)          # (64, 8)
    b2_np = b2v.reshape(NF, 1)

    in_maps = []
    for c in range(NCORES):
        p0, p1 = c * PPC, (c + 1) * PPC
        g = G[nbr[p0:p1]]                         # (2048, 9, 16n, 16b)
        xm_np = np.ascontiguousarray(
            g[:, :8].transpose(1, 2, 0, 3).reshape(128, PPC * B)
        ).astype(_BF16)
        xc_np = np.ascontiguousarray(
            g[:, 8].transpose(1, 0, 2).reshape(16, PPC * B)
        ).astype(_BF16)
        wmc = weight_map[p0:p1]                   # (2048, 9, 16m, 16n)
        wm_np = np.ascontiguousarray(
            wmc[:, :8].transpose(1, 3, 0, 2).reshape(128, PPC * MD)
        ).astype(_BF16)
        wc_np = np.ascontiguousarray(
            wmc[:, 8].transpose(2, 0, 1).reshape(16, PPC * MD)
        ).astype(_BF16)
        nz_np = np.ascontiguousarray(
            noise2[:, p0:p1, :].transpose(2, 1, 0).reshape(8, PPC * B)
        )
        in_maps.append(
            {
                "wm": wm_np, "xm": xm_np, "wc": wc_np, "xc": xc_np,
                "nz": nz_np, "w1t": w1t_np, "b1": b1_np,
                "w2t": w2t_np, "b2": b2_np,
            }
        )

    nc = _get_nc()
    res = run_bass_kernel_spmd(nc, in_maps, core_ids=list(range(NCORES)), trace=TRACE)
    global LAST_RESULTS
    LAST_RESULTS = res

    out = np.empty((B, NF, NPIX), np.float32)
    for c in range(NCORES):
        yc = res.results[c]["yout"].reshape(NF, PPC, B)  # (f, px, b)
        out[:, :, c * PPC : (c + 1) * PPC] = yc.transpose(2, 0, 1)
    return out.reshape(B, NF, H, W)


if __name__ == "__main__":
    sys.path.insert(0, "/root/problem")
    import reference

    inputs = {k: np.asarray(v) for k, v in reference.setup_inputs().items()}
    got = kernel(**inputs)
    exp = np.asarray(reference.reference(**reference.setup_inputs()))
    err = np.abs(got - exp).max() / (np.abs(exp).max() + 1e-9)
    print("rel err:", err)
